# revision 13
# baseline (speedup 1.0000x reference)
"""Trainium2 Bass kernel for nn_NeuralHSMM (8-core SPMD, data-parallel over batch).

Per core: 2 sequences. States live on the 128 partitions throughout.
 - ctx matvecs row-split across cores + AllGather (collective), derived
   HSMM params computed on device.
 - emission log-probs via PE GEMMs; C = cumsum_t(log_b) per state.
 - forward scan: duration logsumexp = free-dim max/exp + fused
   multiply-reduce against rotated exp(logD); transition logsumexp =
   exp-domain PE matmul against A = exp(logA). Normalizer = bulk
   cumsum of per-step max_k log_b, re-anchored to the exact measured
   max every L_MEAS steps via PE transpose (only cross-partition op).
"""
import os
import sys
import numpy as np

sys.path.insert(0, "/opt/trn_rl_repo")

from contextlib import ExitStack

import concourse.bass as bass
import concourse.bacc as bacc
import concourse.mybir as mybir
import concourse.tile as tile

K = 128
DMAX = 48
NF = 256
CDIM = 256
B = 16
T = 768
NCORES = 8
BL = B // NCORES          # 2 sequences per core
TT = BL * T               # 1536
NEG = -1.0e9
LOG2PI = float(np.log(2.0 * np.pi))
L_MEAS = 4
f32 = mybir.dt.float32
AX = mybir.AxisListType
ALU = mybir.AluOpType
ACTF = mybir.ActivationFunctionType

_CACHE = {}


def build_program(ncores=NCORES):
    RA, RD, RE = (K * K) // ncores, (K * DMAX) // ncores, (K * NF) // ncores
    GA, GD, GE = RA // K, RD // K, RE // K
    RTOT = RA + RD + RE

    nc = bacc.Bacc(
        "TRN2",
        target_bir_lowering=False,
        debug=False,
        num_devices=ncores,
    )

    def dp(name, shape, out=False):
        return nc.declare_dram_parameter(name, shape, f32, isOutput=out)

    x_l = dp("x_l", [TT, NF])
    ctx_bc = dp("ctx_bc", [1, CDIM])
    aw_l = dp("aw_l", [RA, CDIM]); ab_l = dp("ab_l", [K, GA])
    dw_l = dp("dw_l", [RD, CDIM]); db_l = dp("db_l", [K, GD])
    ew_l = dp("ew_l", [RE, CDIM]); eb_l = dp("eb_l", [K, GE])
    trans = dp("trans", [K, K])
    dur = dp("dur", [K, DMAX])
    mu_d = dp("mu", [K, NF])
    lv_d = dp("log_var", [K, NF])
    pi_d = dp("pi", [K, 1])
    id_d = dp("ident", [K, K])
    out_p = dp("out_p", [K, BL], out=True)
    out_c = dp("out_c", [BL, 1], out=True)

    cc_in = nc.dram_tensor("cc_in", [RTOT], f32)
    cc_out = nc.dram_tensor("cc_out", [ncores * RTOT], f32, addr_space="Shared")
    groups = [list(range(ncores))]

    with tile.TileContext(nc) as tc, ExitStack() as ctx:
        per = ctx.enter_context(tc.tile_pool(name="per", bufs=1))
        tmp = ctx.enter_context(tc.tile_pool(name="tmp", bufs=2))
        pst = ctx.enter_context(tc.tile_pool(name="pst", bufs=2, space="PSUM"))

        dma = nc.sync.dma_start
        dmag = nc.gpsimd.dma_start

        # ---------- small params ----------
        ident = per.tile([K, K], f32); dma(ident[:], id_d[:])
        ctxb = per.tile([1, CDIM], f32); dma(ctxb[:], ctx_bc[:])
        ctx_bcast = per.tile([K, CDIM], f32)
        nc.gpsimd.partition_broadcast(ctx_bcast[:], ctxb[:])

        # ---------- ctx matvec on this core's row chunk ----------
        parts = per.tile([K, GA + GD + GE], f32)
        GCH = 16  # groups per chunk (SBUF-friendly)
        for o, w_d, b_d, G in ((0, aw_l, ab_l, GA), (GA, dw_l, db_l, GD),
                               (GA + GD, ew_l, eb_l, GE)):
            w3 = w_d[:].rearrange("(p g) c -> p g c", p=K)
            for g0 in range(0, G, GCH):
                gn = min(GCH, G - g0)
                wt = tmp.tile([K, gn * CDIM], f32, tag="wt")
                dma(wt[:], w3[:, g0:g0 + gn, :])
                prod = tmp.tile([K, gn * CDIM], f32, tag="prod")
                nc.vector.tensor_mul(
                    prod[:].rearrange("p (g c) -> p g c", g=gn),
                    wt[:].rearrange("p (g c) -> p g c", g=gn),
                    ctx_bcast[:, None, :].broadcast_to((K, gn, CDIM)))
                nc.vector.tensor_reduce(
                    parts[:, o + g0:o + g0 + gn],
                    prod[:].rearrange("p (g c) -> p g c", g=gn),
                    axis=AX.X, op=ALU.add)
            bt = tmp.tile([K, G], f32, tag="bt")
            dma(bt[:], b_d[:])
            nc.vector.tensor_add(parts[:, o:o + G], parts[:, o:o + G], bt[:])

        dmag(cc_in[:], parts[:])
        if ncores > 1:
            nc.gpsimd.collective_compute(
                "AllGather", ALU.bypass, replica_groups=groups,
                ins=[cc_in[:]], outs=[cc_out[:]])
        else:
            dmag(cc_out[:], cc_in[:])

        cc3 = cc_out[:].rearrange("(r x) -> r x", x=RTOT)
        aA = per.tile([K, K], f32)
        dma(aA[:], cc3[:, 0:RA])
        aD = per.tile([K, DMAX], f32)
        dma(aD[:], cc3[:, RA:RA + RD])
        aE = per.tile([K, NF], f32)
        dma(aE[:], cc3[:, RA + RD:RTOT])

        # ---------- derived params ----------
        def tanh01_add(logits_d, a_t, width):
            # z = logits + 0.1*tanh(a) = logits + 0.1 - 0.2/(exp(2a)+1)
            e2 = tmp.tile([K, width], f32, tag="e2" + str(width))
            nc.scalar.activation(e2[:], a_t[:], ACTF.Exp, scale=2.0)
            nc.vector.tensor_scalar_add(e2[:], e2[:], 1.0)
            rc = tmp.tile([K, width], f32, tag="rc" + str(width))
            nc.vector.reciprocal(rc[:], e2[:])
            lg = tmp.tile([K, width], f32, tag="lg" + str(width))
            dma(lg[:], logits_d[:])
            nc.vector.tensor_scalar_add(lg[:], lg[:], 0.1)
            z = tmp.tile([K, width], f32, tag="zz" + str(width))
            nc.vector.scalar_tensor_tensor(
                z[:], rc[:], -0.2, lg[:], op0=ALU.mult, op1=ALU.add)
            return z

        def row_softmax_exp(z, width, out_tile):
            # out = exp(z - max - log(sum exp(z - max)))
            mxn = tmp.tile([K, 1], f32, tag="smx" + str(width))
            nc.vector.tensor_reduce(mxn[:], z[:], axis=AX.X, op=ALU.max,
                                    negate=True)
            nc.vector.tensor_scalar_add(z[:], z[:], mxn[:])
            ez = tmp.tile([K, width], f32, tag="sez" + str(width))
            nc.scalar.activation(ez[:], z[:], ACTF.Exp)
            sme = tmp.tile([K, 1], f32, tag="ssm" + str(width))
            nc.vector.tensor_reduce(sme[:], ez[:], axis=AX.X, op=ALU.add)
            lsm = tmp.tile([K, 1], f32, tag="sls" + str(width))
            nc.scalar.activation(lsm[:], sme[:], ACTF.Ln)
            nc.vector.tensor_scalar_mul(lsm[:], lsm[:], -1.0)
            nc.scalar.activation(out_tile[:], z[:], ACTF.Exp, bias=lsm[:])

        A_sb = per.tile([K, K], f32)
        row_softmax_exp(tanh01_add(trans, aA, K), K, A_sb)
        Dhat = per.tile([K, DMAX], f32)
        row_softmax_exp(tanh01_add(dur, aD, DMAX), DMAX, Dhat)

        mu_sb = tmp.tile([K, NF], f32, tag="mu")
        dma(mu_sb[:], mu_d[:])
        mu_eff = per.tile([K, NF], f32)
        nc.vector.scalar_tensor_tensor(
            mu_eff[:], aE[:], 0.1, mu_sb[:], op0=ALU.mult, op1=ALU.add)

        lv = tmp.tile([K, NF], f32, tag="lv")
        dma(lv[:], lv_d[:])
        ab2 = tmp.tile([K, NF], f32, tag="ab2")
        nc.scalar.activation(ab2[:], lv[:], ACTF.Abs)
        en = tmp.tile([K, NF], f32, tag="en")
        nc.scalar.activation(en[:], ab2[:], ACTF.Exp, scale=-1.0)
        l1 = tmp.tile([K, NF], f32, tag="l1")
        nc.scalar.activation(l1[:], en[:], ACTF.Ln, bias=1.0)
        rl = tmp.tile([K, NF], f32, tag="rl")
        nc.scalar.activation(rl[:], lv[:], ACTF.Relu)
        var = per.tile([K, NF], f32)
        nc.vector.tensor_add(var[:], rl[:], l1[:])
        nc.vector.tensor_scalar_add(var[:], var[:], 1e-3)
        inv = per.tile([K, NF], f32)
        nc.vector.reciprocal(inv[:], var[:])
        lnv = tmp.tile([K, NF], f32, tag="lnv")
        nc.scalar.activation(lnv[:], var[:], ACTF.Ln)
        lnvs = tmp.tile([K, 1], f32, tag="lnvs")
        nc.vector.tensor_reduce(lnvs[:], lnv[:], axis=AX.X, op=ALU.add)
        M2 = per.tile([K, NF], f32)
        nc.vector.scalar_tensor_tensor(
            M2[:], mu_eff[:], -2.0, inv[:], op0=ALU.mult, op1=ALU.mult)
        s1scr = tmp.tile([K, NF], f32, tag="s1scr")
        s1 = tmp.tile([K, 1], f32, tag="s1")
        nc.vector.scalar_tensor_tensor(
            s1scr[:], mu_eff[:], 1.0, M2[:], op0=ALU.mult, op1=ALU.mult,
            accum_out=s1[:])
        bias_k = per.tile([K, 1], f32)
        nc.vector.tensor_scalar_mul(s1[:], s1[:], 0.25)
        nc.vector.scalar_tensor_tensor(
            bias_k[:], lnvs[:], -0.5, s1[:], op0=ALU.mult, op1=ALU.add)
        nc.vector.tensor_scalar_add(bias_k[:], bias_k[:], -NF * LOG2PI / 2.0)

        M1T = per.tile([K, NF], f32)
        M2T = per.tile([K, NF], f32)
        for c in range(2):
            for src, dst in ((inv, M1T), (M2, M2T)):
                pp = pst.tile([K, K], f32, tag="ps")
                nc.tensor.transpose(pp[:], src[:, c * K:(c + 1) * K], ident[:])
                nc.vector.tensor_copy(dst[:, c * K:(c + 1) * K], pp[:])

        # ---------- logpi (transposed-space softmax via PE) ----------
        pi_sb = tmp.tile([K, 1], f32, tag="pi")
        dma(pi_sb[:], pi_d[:])
        piP = pst.tile([1, K], f32, tag="ps", name="piP")
        nc.tensor.transpose(piP[:], pi_sb[:], ident[:])
        zp = tmp.tile([1, K], f32, tag="zpT")
        mxp = tmp.tile([1, 1], f32, tag="mxp")
        nc.vector.tensor_reduce(mxp[:], piP[:], axis=AX.X, op=ALU.max,
                                negate=True)
        nc.vector.tensor_scalar_add(zp[:], piP[:], mxp[:])
        ep = tmp.tile([1, K], f32, tag="ep")
        nc.scalar.activation(ep[:], zp[:], ACTF.Exp)
        smp = tmp.tile([1, 1], f32, tag="smp")
        nc.vector.tensor_reduce(smp[:], ep[:], axis=AX.X, op=ALU.add)
        lsp = tmp.tile([1, 1], f32, tag="lsp")
        nc.scalar.activation(lsp[:], smp[:], ACTF.Ln)
        nc.vector.tensor_scalar_mul(lsp[:], lsp[:], -1.0)
        nc.vector.tensor_scalar_add(zp[:], zp[:], lsp[:])  # = logpi^T [1,K]
        lpP = pst.tile([K, 1], f32, tag="ps", name="lpP")
        nc.tensor.transpose(lpP[:], zp[:], ident[0:1, 0:1])
        lpT = per.tile([K, 1], f32)
        nc.vector.tensor_copy(lpT[:], lpP[:])
        mxlp = tmp.tile([1, 2], f32, tag="mxlp")
        nc.vector.tensor_reduce(mxlp[:, 0:1], zp[:], axis=AX.X, op=ALU.max)
        nc.vector.tensor_copy(mxlp[:, 1:2], mxlp[:, 0:1])

        # ---------- emissions ----------
        NT = TT // K
        with tc.tile_pool(name="em", bufs=3) as em, \
             tc.tile_pool(name="emp", bufs=2, space="PSUM") as emp:
            xT = [per.tile([K, TT], f32, name=f"xT{c}", tag=f"xT{c}") for c in range(2)]
            sqT = [per.tile([K, TT], f32, name=f"sqT{c}", tag=f"sqT{c}") for c in range(2)]
            for r in range(NT):
                xt = em.tile([K, NF], f32, tag="xt")
                dma(xt[:], x_l[r * K:(r + 1) * K, :])
                for c in range(2):
                    pp = emp.tile([K, K], f32, tag="em")
                    nc.tensor.transpose(pp[:], xt[:, c * K:(c + 1) * K],
                                        ident[:])
                    nc.vector.tensor_copy(xT[c][:, r * K:(r + 1) * K], pp[:])
                    nc.scalar.activation(sqT[c][:, r * K:(r + 1) * K], pp[:],
                                         ACTF.Square)
            log_b = per.tile([K, TT], f32)
            for b_ in range(TT // 512):
                sl = slice(b_ * 512, (b_ + 1) * 512)
                acc = emp.tile([K, 512], f32, tag="em", name="acc")
                nc.tensor.matmul(acc[:], M1T[:, 0:K], sqT[0][:, sl],
                                 start=True, stop=False)
                nc.tensor.matmul(acc[:], M1T[:, K:NF], sqT[1][:, sl],
                                 start=False, stop=False)
                nc.tensor.matmul(acc[:], M2T[:, 0:K], xT[0][:, sl],
                                 start=False, stop=False)
                nc.tensor.matmul(acc[:], M2T[:, K:NF], xT[1][:, sl],
                                 start=False, stop=True)
                nc.scalar.activation(log_b[:, sl], acc[:], ACTF.Identity,
                                     scale=-0.5, bias=bias_k[:])

            dcols = per.tile([K, NT], f32)
            for r in range(NT):
                pp = emp.tile([K, K], f32, tag="em")
                nc.tensor.transpose(pp[:], log_b[:, r * K:(r + 1) * K],
                                    ident[:])
                nc.vector.tensor_reduce(dcols[:, r:r + 1], pp[:], axis=AX.X,
                                        op=ALU.max)
            dP = emp.tile([NT, K], f32, tag="em", name="dP")
            nc.tensor.transpose(dP[:], dcols[:], ident[:])
            dT = per.tile([NT, K], f32)
            nc.vector.tensor_copy(dT[:], dP[:])

        d2 = per.tile([BL, T], f32)
        for s in range(BL):
            for b_ in range(T // K):
                dma(d2[s:s + 1, b_ * K:(b_ + 1) * K],
                    dT[s * (T // K) + b_:s * (T // K) + b_ + 1, :])
        cum2 = per.tile([BL, T], f32)
        zb2 = per.tile([BL, T], f32)
        nc.vector.memset(zb2[:], 0.0)
        nc.vector.tensor_tensor_scan(cum2[:], d2[:], zb2[:], 0.0,
                                     op0=ALU.add, op1=ALU.add)

        C_il = per.tile([K, TT], f32)
        zbT = per.tile([K, T], f32)
        nc.vector.memset(zbT[:], 0.0)
        for s in range(BL):
            nc.vector.tensor_tensor_scan(
                C_il[:, s::2], log_b[:, s * T:(s + 1) * T], zbT[:], 0.0,
                op0=ALU.add, op1=ALU.add)
        cum2b = per.tile([1, T], f32)
        dma(cum2b[:], cum2[1:2, :])
        cumb = per.tile([K, TT], f32)
        nc.gpsimd.partition_broadcast(cumb[:, 0::2], cum2[0:1, :])
        nc.gpsimd.partition_broadcast(cumb[:, 1::2], cum2b[:])
        CC = per.tile([K, TT], f32)
        nc.vector.tensor_sub(CC[:], C_il[:], cumb[:])

        # ---------- Dhat rotations (pair-duplicated, reversed windows) ----------
        REV2 = per.tile([K, 192], f32)
        for i in range(96):
            csrc = (47 - i) % 48
            nc.vector.tensor_copy(
                REV2[:, 2 * i:2 * i + 2],
                Dhat[:, csrc:csrc + 1].broadcast_to((K, 2)))
        Drot = per.tile([K, 48 * 96], f32)
        for c in range(48):
            nc.vector.tensor_copy(Drot[:, c * 96:(c + 1) * 96],
                                  REV2[:, 2 * c:2 * c + 96])

        # ---------- scan state init ----------
        rb = per.tile([K, 2 * DMAX], f32)
        nc.vector.memset(rb[:], NEG)
        nc.vector.tensor_copy(rb[:, 2 * DMAX - 2:2 * DMAX],
                              lpT[:, 0:1].broadcast_to((K, 2)))
        SM = per.tile([K, 2], f32)
        P = per.tile([K, 2], f32)
        OUTC = per.tile([BL, 1], f32)
        ZERO2 = per.tile([2, K], f32)
        nc.vector.memset(ZERO2[:], 0.0)
        BT = [per.tile([2, 1], f32, name=f"BT{j}", tag=f"BT{j}") for j in range(2)]
        TB = [per.tile([K, 2], f32, name=f"TB{j}", tag=f"TB{j}") for j in range(2)]
        nc.gpsimd.partition_broadcast(BT[0][:], mxlp[:, 0:1])
        nc.gpsimd.partition_broadcast(TB[0][:], mxlp[:])

        # ---------- scan ----------
        loop = ctx.enter_context(tc.tile_pool(name="loop", bufs=4))
        qpool = ctx.enter_context(tc.tile_pool(name="qp", bufs=4, space="PSUM"))
        tpool = ctx.enter_context(tc.tile_pool(name="tp", bufs=2, space="PSUM"))

        def step(CCW, c2w, u):
            cur = (u // L_MEAS) % 2
            c = (48 - u) % 48
            MXN = loop.tile([K, 2], f32, tag="MXN")
            nc.vector.tensor_reduce(
                MXN[:], rb[:].rearrange("p (j s) -> p s j", s=2),
                axis=AX.X, op=ALU.max, negate=True)
            X = loop.tile([K, 98], f32, tag="X")
            for s in range(2):
                nc.vector.tensor_scalar_add(
                    X[:, s:96:2], rb[:, s::2], MXN[:, s:s + 1])
            U1 = loop.tile([K, 2], f32, tag="U1")
            nc.vector.tensor_sub(U1[:], CCW[:, 2 * u:2 * u + 2], TB[cur][:])
            nc.vector.tensor_sub(X[:, 96:98], U1[:], MXN[:])
            E = loop.tile([K, 98], f32, tag="E")
            nc.scalar.activation(E[:], X[:], ACTF.Exp)
            scr = loop.tile([K, 96], f32, tag="scr")
            for s in range(2):
                nc.vector.scalar_tensor_tensor(
                    scr[:, s::2], E[:, s:96:2], 1.0,
                    Drot[:, c * 96 + s:(c + 1) * 96:2],
                    op0=ALU.mult, op1=ALU.mult,
                    accum_out=SM[:, s:s + 1])
            nc.vector.tensor_mul(P[:], SM[:], E[:, 96:98])
            QP = qpool.tile([K, 2], f32, tag="QP")
            nc.tensor.matmul(QP[:], A_sb[:], P[:], start=True, stop=True)
            LQ = loop.tile([K, 2], f32, tag="LQ")
            nc.scalar.activation(LQ[:], QP[:], ACTF.Ln)
            nc.vector.tensor_sub(rb[:, 2 * u:2 * u + 2], LQ[:], U1[:])
            if u == DMAX - 1:
                nc.vector.tensor_add(OUTC[:], BT[cur][:], c2w[:, u:u + 1])
            if u % L_MEAS == L_MEAS - 1:
                nxt = 1 - cur
                SP1 = tpool.tile([2, K], f32, tag="tp", name="SP1")
                nc.tensor.transpose(SP1[:], X[:, 96:98], ident[:])
                CRED = loop.tile([2, 1], f32, tag="CRED")
                nc.vector.tensor_reduce(CRED[:], SP1[:], axis=AX.X, op=ALU.max)
                nc.vector.tensor_add(BT[nxt][:], BT[cur][:], CRED[:])
                TIN = loop.tile([2, K], f32, tag="TIN")
                nc.vector.tensor_scalar_add(TIN[:], ZERO2[:], BT[nxt][:])
                SP2 = tpool.tile([K, 2], f32, tag="tp", name="SP2")
                nc.tensor.transpose(SP2[:], TIN[:], ident[0:2, 0:2])
                nc.vector.tensor_copy(TB[nxt][:], SP2[:])

        with tc.For_i(0, T // DMAX, 1) as it:
            CCW = loop.tile([K, 2 * DMAX], f32, tag="CCW")
            nc.vector.tensor_copy(
                CCW[:], CC[:, bass.ds(2 * DMAX * it, 2 * DMAX)])
            c2w = loop.tile([BL, DMAX], f32, tag="c2w")
            nc.vector.tensor_copy(c2w[:], cum2[:, bass.ds(DMAX * it, DMAX)])
            for u in range(DMAX):
                step(CCW, c2w, u)

        dma(out_p[:], P[:])
        dma(out_c[:], OUTC[:])

    nc.finalize()
    return nc


def _get_program(ncores=NCORES):
    if ncores not in _CACHE:
        _CACHE[ncores] = build_program(ncores)
    return _CACHE[ncores]


def make_in_maps(inputs, ncores=NCORES):
    f = lambda a: np.ascontiguousarray(np.asarray(a), dtype=np.float32)
    x = f(inputs["x"])
    ctxv = f(inputs["context"]).reshape(1, CDIM)
    aw, ab = f(inputs["ctx_A_w"]), f(inputs["ctx_A_b"])
    dw, db = f(inputs["ctx_D_w"]), f(inputs["ctx_D_b"])
    ew, eb = f(inputs["ctx_E_w"]), f(inputs["ctx_E_b"])
    RA, RD, RE = (K * K) // ncores, (K * DMAX) // ncores, (K * NF) // ncores
    ident = np.eye(K, dtype=np.float32)
    common = {
        "ctx_bc": ctxv,
        "trans": f(inputs["trans_logits"]),
        "dur": f(inputs["dur_logits"]),
        "mu": f(inputs["mu"]),
        "log_var": f(inputs["log_var"]),
        "pi": f(inputs["pi_logits"]).reshape(K, 1),
        "ident": ident,
    }
    maps = []
    for cix in range(ncores):
        m = dict(common)
        m["x_l"] = np.ascontiguousarray(
            x[cix * BL:(cix + 1) * BL].reshape(TT, NF))
        m["aw_l"] = np.ascontiguousarray(aw[cix * RA:(cix + 1) * RA])
        m["ab_l"] = np.ascontiguousarray(ab[cix * RA:(cix + 1) * RA].reshape(K, -1))
        m["dw_l"] = np.ascontiguousarray(dw[cix * RD:(cix + 1) * RD])
        m["db_l"] = np.ascontiguousarray(db[cix * RD:(cix + 1) * RD].reshape(K, -1))
        m["ew_l"] = np.ascontiguousarray(ew[cix * RE:(cix + 1) * RE])
        m["eb_l"] = np.ascontiguousarray(eb[cix * RE:(cix + 1) * RE].reshape(K, -1))
        maps.append(m)
    return maps


def assemble_output(results):
    out = np.empty(B, np.float32)
    for cix, r in enumerate(results):
        p = np.asarray(r["out_p"], np.float32)      # [K, BL]
        cch = np.asarray(r["out_c"], np.float32)    # [BL, 1]
        for s in range(BL):
            out[cix * BL + s] = cch[s, 0] + np.float32(
                np.log(p[:, s].sum(dtype=np.float32)))
    return out


def kernel(**inputs):
    from concourse.bass_utils import run_bass_kernel_spmd
    nc = _get_program(NCORES)
    in_maps = make_in_maps(inputs, NCORES)
    res = run_bass_kernel_spmd(nc, in_maps, list(range(NCORES)))
    return assemble_output(res.results)
